# revision 1
# baseline (speedup 1.0000x reference)
"""Trainium2 Bass kernel for MultiHeadedAttentionTree (relative-position tree attention).

Problem (hardcoded): B=4, S=512, D=512, H=8, dk=64, NUM_REL=41, f32.

  q = query@Wq+bq; k = key@Wk+bk; v = value@Wv+bv  (split into 8 heads of 64)
  scores = (q.k^T + q.emb_k[mask]^T) * (1/8);  scores[mask==0] = -inf
  attn = softmax(scores); x = attn@v + attn@emb_v[mask];  out = x@Wo + bo

Sharding: 8 NeuronCores = 4 batches x 2 query-halves. Each core gets
query/mask rows for its (batch, q-half), full key/value for its batch, and
replicated weights/embeddings. Outputs are disjoint row-blocks of the result.

Algorithm per core (all PE matmuls bf16, f32 PSUM accumulation):
  - Load + cast inputs to bf16; PE-transpose query/key/value.
  - Projections produce qT/kT (transposed, [D,tok]) and v ([k,D]).
  - Content scores per head: s = qT_h^T @ kT_h -> [q,k] PSUM, evac *scale to bf16.
  - Rel tables p[h,q,r] = q_h . emb_k[r] * scale, with slot r=0 overwritten
    by -50 (implements the mask>0 masking inside the softmax; exp(-50+smax)~0).
  - eq_r = (mask == r) planes, then per (r,h): s += eq_r * p[:,r] (fused
    scalar_tensor_tensor MAC, split across Vector and GpSimd engines).
  - w = exp(s) via ScalarE with row-sum accumulator -> denominator; w *= 1/den.
  - Value side: per (r,h) fused multiply+row-reduce gives bucket sums
    Wb[h,q,r] = sum_k w*eq_r; then x^T = v_h^T @ w_h^T + emb_v^T @ Wb_h^T on PE.
  - out = x@Wo + bo via lhsT = x^T chunks; DMA out f32.
"""

import numpy as np
from contextlib import ExitStack

import concourse.bass as bass
import concourse.tile as tile
import concourse.mybir as mybir
from concourse import bacc
from concourse.bass_utils import run_bass_kernel_spmd

dt = mybir.dt
Alu = mybir.AluOpType
Act = mybir.ActivationFunctionType

B, S, D, H = 4, 512, 512, 8
DK = D // H          # 64
R = 41               # relative-position vocabulary
QS = S // 2          # 256 queries per core
SCALE = 1.0 / 8.0    # 1/sqrt(dk)
NEGB = -50.0         # pre-scaled "minus infinity" for masked slots
N_CORES = 8

_cached_nc = None


def _transpose_in(nc, tc, pools, dst_tiles, src_tile, n_row_tiles, n_col_tiles,
                  identb, dst_col0):
    """PE-transpose src_tile [128*n_row_tiles? ...] helper — see call sites."""
    raise NotImplementedError


def build_nc(dummy=False):
    nc = bacc.Bacc("TRN2", target_bir_lowering=False, debug=False,
                   num_devices=N_CORES)

    # ---------------- I/O ----------------
    query_d = nc.dram_tensor("query_s", [QS, D], dt.float32, kind="ExternalInput")
    key_d = nc.dram_tensor("key_s", [S, D], dt.float32, kind="ExternalInput")
    value_d = nc.dram_tensor("value_s", [S, D], dt.float32, kind="ExternalInput")
    mask_d = nc.dram_tensor("mask_s", [QS, S], dt.int32, kind="ExternalInput")
    Wq_d = nc.dram_tensor("Wq", [D, D], dt.float32, kind="ExternalInput")
    Wk_d = nc.dram_tensor("Wk", [D, D], dt.float32, kind="ExternalInput")
    Wv_d = nc.dram_tensor("Wv", [D, D], dt.float32, kind="ExternalInput")
    Wo_d = nc.dram_tensor("Wo", [D, D], dt.float32, kind="ExternalInput")
    bq_d = nc.dram_tensor("bq", [D], dt.float32, kind="ExternalInput")
    bk_d = nc.dram_tensor("bk", [D], dt.float32, kind="ExternalInput")
    bv_d = nc.dram_tensor("bv", [D], dt.float32, kind="ExternalInput")
    bo_d = nc.dram_tensor("bo", [D], dt.float32, kind="ExternalInput")
    ek_d = nc.dram_tensor("emb_k", [R, DK], dt.float32, kind="ExternalInput")
    ev_d = nc.dram_tensor("emb_v", [R, DK], dt.float32, kind="ExternalInput")
    out_d = nc.dram_tensor("out_s", [QS, D], dt.float32, kind="ExternalOutput")

    identb_d = nc.inline_tensor(np.eye(128, dtype=np.float32), name="identb")

    NT = S // 128          # 4 token tiles for key/value
    NQT = QS // 128        # 2 query tiles
    KC = S // 128          # 4 k-chunks
    DC = D // 128          # 4 D-chunks

    if dummy:
        with tile.TileContext(nc) as tc, ExitStack() as ctx:
            pool = ctx.enter_context(tc.tile_pool(name="dp", bufs=2))
            for t in range(QS // 128):
                ti = pool.tile([128, D], dt.float32, name="dtile", tag="dtile")
                nc.sync.dma_start(ti[:], query_d[t * 128:(t + 1) * 128, :])
                nc.sync.dma_start(out_d[t * 128:(t + 1) * 128, :], ti[:])
        nc.compile()
        return nc

    with tile.TileContext(nc) as tc, ExitStack() as ctx:
        persist = ctx.enter_context(tc.tile_pool(name="persist", bufs=1))
        io_pool = ctx.enter_context(tc.tile_pool(name="io", bufs=3))
        ev_pool = ctx.enter_context(tc.tile_pool(name="evac", bufs=3))
        ps_mm = ctx.enter_context(tc.tile_pool(name="psmm", bufs=2, space="PSUM"))
        ps_tr = ctx.enter_context(tc.tile_pool(name="pstr", bufs=4, space="PSUM"))
        ps_x = ctx.enter_context(tc.tile_pool(name="psx", bufs=2, space="PSUM"))
        trash_pool = ctx.enter_context(tc.tile_pool(name="trash", bufs=2))

        # ---------------- load + cast ----------------
        identf = persist.tile([128, 128], dt.float32, name="identf", tag="identf")
        nc.sync.dma_start(identf[:], identb_d[:])
        identb = persist.tile([128, 128], dt.bfloat16, name="identb", tag="identb")
        nc.vector.tensor_copy(identb[:], identf[:])

        def load_cast_rows(src_d, rows, tag):
            """Load [rows, D] f32 from DRAM, return list of [128, D] bf16 tiles."""
            tiles = []
            for t in range(rows // 128):
                f = io_pool.tile([128, D], dt.float32, name="ldf32", tag="ldf32")
                nc.sync.dma_start(f[:], src_d[t * 128:(t + 1) * 128, :])
                bfti = persist.tile([128, D], dt.bfloat16, name=f"{tag}{t}", tag=f"{tag}{t}")
                nc.vector.tensor_copy(bfti[:], f[:])
                tiles.append(bfti)
            return tiles

        query_bf = load_cast_rows(query_d, QS, "query_bf")
        key_bf = load_cast_rows(key_d, S, "key_bf")
        value_bf = load_cast_rows(value_d, S, "value_bf")
        Wq_bf = load_cast_rows(Wq_d, D, "Wq_bf")
        Wk_bf = load_cast_rows(Wk_d, D, "Wk_bf")
        Wv_bf = load_cast_rows(Wv_d, D, "Wv_bf")
        Wo_sl = []
        for h in range(H):
            f = io_pool.tile([DK, D], dt.float32, name="ldwo", tag="ldf32")
            nc.sync.dma_start(f[:DK, :], Wo_d[h * DK:(h + 1) * DK, :])
            o = persist.tile([DK, D], dt.bfloat16, name=f"Wo_sl{h}", tag=f"Wo_sl{h}")
            nc.vector.tensor_copy(o[:], f[:DK, :])
            Wo_sl.append(o)

        # mask -> bf16 [128, S] per q-tile
        mask_bf = []
        for t in range(NQT):
            mi = io_pool.tile([128, S], dt.int32, name="mask_i32", tag="mask_i32")
            nc.sync.dma_start(mi[:], mask_d[t * 128:(t + 1) * 128, :])
            mb = persist.tile([128, S], dt.bfloat16, name=f"mask_bf{t}", tag=f"mask_bf{t}")
            nc.vector.tensor_copy(mb[:], mi[:])
            mask_bf.append(mb)

        # biases as [128,1] column chunks: bcol[c][p] = b[c*128+p]
        def load_bias_cols(src_d, tag):
            cols = persist.tile([128, DC], dt.float32, tag=tag)
            nc.sync.dma_start(
                cols[:], src_d.rearrange("(c p) -> p c", p=128))
            return cols

        bq_c = load_bias_cols(bq_d, "bq_c")
        bk_c = load_bias_cols(bk_d, "bk_c")
        # bv, bo as [1, D] rows for rank-1 matmul adds
        bv_row = persist.tile([1, D], dt.bfloat16, name="bv_row", tag="bv_row")
        bv_f = io_pool.tile([1, D], dt.float32, name="bias_row_f", tag="bias_row_f")
        nc.sync.dma_start(bv_f[:], bv_d.rearrange("(a d) -> a d", a=1))
        nc.vector.tensor_copy(bv_row[:], bv_f[:])
        bo_row = persist.tile([1, D], dt.bfloat16, name="bo_row", tag="bo_row")
        bo_f = io_pool.tile([1, D], dt.float32, name="bias_row_f", tag="bias_row_f")
        nc.sync.dma_start(bo_f[:], bo_d.rearrange("(a d) -> a d", a=1))
        nc.vector.tensor_copy(bo_row[:], bo_f[:])
        ones_col = persist.tile([1, 128], dt.bfloat16, name="ones_col", tag="ones_col")
        nc.vector.memset(ones_col[:], 1.0)

        # emb_k: [41, 64] -> bf16, duplicated into cols 0:64 and 64:128, then
        # transposed so BOTH 64-row halves of ekT hold emb_k^T (for head slabs)
        ek_pad = persist.tile([128, 128], dt.bfloat16, name="ek_pad", tag="ek_pad")
        nc.vector.memset(ek_pad[:], 0.0)
        ekf = io_pool.tile([R, DK], dt.float32, name="emb_f", tag="emb_f")
        nc.sync.dma_start(ekf[:], ek_d[:])
        nc.vector.tensor_copy(ek_pad[:R, :DK], ekf[:])
        nc.vector.tensor_copy(ek_pad[:R, DK:2 * DK], ekf[:])
        ekT_ps = ps_tr.tile([128, 128], dt.bfloat16, name="embT_ps", tag="trps")
        nc.tensor.transpose(ekT_ps[:], ek_pad[:], identb[:])
        ekT = persist.tile([128, 128], dt.bfloat16, name="ekT", tag="ekT")
        nc.vector.tensor_copy(ekT[:], ekT_ps[:])  # rows 0:64 and 64:128 = emb_k^T

        ev_bf = persist.tile([R, DK], dt.bfloat16, name="ev_bf", tag="ev_bf")
        evf = io_pool.tile([R, DK], dt.float32, name="emb_f", tag="emb_f")
        nc.sync.dma_start(evf[:], ev_d[:])
        nc.vector.tensor_copy(ev_bf[:], evf[:])

        # ---------------- input transposes (PE) ----------------
        def transpose_rows_to_T(src_tiles, n_tok_tiles, width, tag):
            """src: list of [128, D] bf16 (token-major). Returns list of DC
            tiles [128, width] bf16 holding the transpose [D, tok]."""
            out = [persist.tile([128, width], dt.bfloat16, name=f"{tag}{c}", tag=f"{tag}{c}")
                   for c in range(DC)]
            for t in range(n_tok_tiles):
                for c in range(DC):
                    tp = ps_tr.tile([128, 128], dt.bfloat16, name="trp", tag="trps")
                    nc.tensor.transpose(
                        tp[:], src_tiles[t][:, c * 128:(c + 1) * 128], identb[:])
                    nc.scalar.copy(
                        out[c][:, t * 128:(t + 1) * 128], tp[:])
            return out

        queryT = transpose_rows_to_T(query_bf, NQT, QS, "queryT")   # [t, q]
        keyT = transpose_rows_to_T(key_bf, NT, S, "keyT")           # [t, k]
        valueT = transpose_rows_to_T(value_bf, NT, S, "valueT")     # [t, k]

        # ---------------- projections ----------------
        # qT[d, q] = sum_t Wq[t, d] * queryT[t, q]  (+ bq[d])
        def project_T(W_tiles, xT_tiles, bias_cols, width, tag):
            out = []
            for c in range(DC):
                ps = ps_mm.tile([128, width], dt.float32, name="projps", tag="mmps")
                for t in range(DC):
                    nc.tensor.matmul(
                        ps[:], W_tiles[t][:, c * 128:(c + 1) * 128],
                        xT_tiles[t][:, :width],
                        start=(t == 0), stop=(t == DC - 1))
                o = persist.tile([128, width], dt.bfloat16, name=f"{tag}{c}", tag=f"{tag}{c}")
                nc.vector.tensor_scalar(
                    out=o[:], in0=ps[:], scalar1=bias_cols[:, c:c + 1],
                    scalar2=None, op0=Alu.add)
                out.append(o)
            return out

        qT = project_T(Wq_bf, queryT, bq_c, QS, "qT")   # [D, q] chunks
        kT = project_T(Wk_bf, keyT, bk_c, S, "kT")      # [D, k] chunks

        # v[k, d] = sum_t value[k, t] * Wv[t, d] + bv[d]: lhsT = valueT chunks
        vv = []
        for kc in range(KC):
            ps = ps_mm.tile([128, D], dt.float32, name="projps", tag="mmps")
            for t in range(DC):
                nc.tensor.matmul(
                    ps[:], valueT[t][:, kc * 128:(kc + 1) * 128], Wv_bf[t][:],
                    start=(t == 0), stop=False)
            nc.tensor.matmul(ps[:], ones_col[:], bv_row[:],
                             start=False, stop=True)
            o = persist.tile([128, D], dt.bfloat16, name=f"vv{kc}", tag=f"vv{kc}")
            nc.vector.tensor_copy(o[:], ps[:])
            vv.append(o)

        # ---------------- rel tables p[h][q, r] ----------------
        # p = (q_h . emb_k_r) * scale, then p[:, 0] = NEGB
        p_tiles = {}
        p_tiles_bf = {}
        for qt in range(NQT):
            for h in range(H):
                c, off = h // 2, (h % 2) * DK
                ps = ps_mm.tile([128, R], dt.float32, name="pps", tag="mmps")
                nc.tensor.matmul(
                    ps[:],
                    qT[c][off:off + DK, qt * 128:(qt + 1) * 128],
                    ekT[off:off + DK, :R],
                    start=True, stop=True)
                p_sb = persist.tile([128, R], dt.float32, name=f"p{qt}_{h}", tag=f"p{qt}_{h}")
                nc.vector.tensor_scalar(
                    out=p_sb[:], in0=ps[:], scalar1=SCALE, scalar2=None,
                    op0=Alu.mult)
                nc.vector.memset(p_sb[:, 0:1], NEGB)
                p_tiles[(qt, h)] = p_sb
                p_bf = persist.tile([128, R], dt.bfloat16, name=f"pb{qt}_{h}", tag=f"pb{qt}_{h}")
                nc.vector.tensor_copy(p_bf[:], p_sb[:])
                p_tiles_bf[(qt, h)] = p_bf

        # ---------------- per-q-tile attention ----------------
        out_f32 = persist.tile([128, D], dt.float32, name="out_f32", tag="out_f32")

        for qt in range(NQT):
            # content-exp c = exp(s_c * scale) straight out of PSUM via ACT,
            # and zero-initialized rel tiles for the MAC loop
            cexp = []
            stot = []
            for h in range(H):
                c, off = h // 2, (h % 2) * DK
                ps = ps_mm.tile([128, S], dt.float32, name="scps", tag="mmps")
                nc.tensor.matmul(
                    ps[:],
                    qT[c][off:off + DK, qt * 128:(qt + 1) * 128],
                    kT[c][off:off + DK, :],
                    start=True, stop=True)
                ce = persist.tile([128, S], dt.bfloat16, name=f"cexp{h}", tag=f"cexp{h}")
                nc.scalar.activation(ce[:], ps[:], Act.Exp, bias=0.0, scale=SCALE)
                cexp.append(ce)
                sb = persist.tile([128, S], dt.bfloat16, name=f"stot{h}", tag=f"stot{h}")
                nc.gpsimd.memset(sb[:], 0.0)
                stot.append(sb)

            # eq planes for all 41 r values
            eq_all = persist.tile([128, R, S], dt.bfloat16, name="eq_all", tag="eq_all")
            for r in range(R):
                nc.vector.tensor_scalar(
                    out=eq_all[:, r, :], in0=mask_bf[qt][:], scalar1=float(r),
                    scalar2=None, op0=Alu.is_equal)

            # score MACs: stot[h] += eq_r * p[h][:, r]
            # heads 0..5 run as mul+add pairs on GpSimd, 6..7 fused on Vector
            GP_H = 6
            for r in range(R):
                for h in range(H):
                    if h < GP_H:
                        pbc = p_tiles_bf[(qt, h)][:, r:r + 1].broadcast_to((128, S))
                        tmp = trash_pool.tile([128, S], dt.bfloat16,
                                              name="gtmp", tag=f"gtmp{h % 3}")
                        nc.gpsimd.tensor_tensor(tmp[:], eq_all[:, r, :], pbc,
                                                Alu.mult)
                        nc.gpsimd.tensor_tensor(stot[h][:], stot[h][:], tmp[:],
                                                Alu.add)
                    else:
                        nc.vector.scalar_tensor_tensor(
                            out=stot[h][:], in0=eq_all[:, r, :],
                            scalar=p_tiles[(qt, h)][:, r:r + 1],
                            in1=stot[h][:], op0=Alu.mult, op1=Alu.add)

            # softmax: F = exp(rel) on ACT, w = c*F fused with denom accum,
            # then normalize by 1/denom (no max subtraction: scores ~ +-8)
            wn = []
            for h in range(H):
                fex = ev_pool.tile([128, S], dt.bfloat16, name="fex", tag="fex")
                nc.scalar.activation(fex[:], stot[h][:], Act.Exp,
                                     bias=0.0, scale=1.0)
                den = ev_pool.tile([128, 1], dt.float32, name="den", tag="den")
                w = persist.tile([128, S], dt.bfloat16, name=f"w{h}", tag=f"w{h}")
                nc.vector.scalar_tensor_tensor(
                    out=w[:], in0=cexp[h][:], scalar=1.0, in1=fex[:],
                    op0=Alu.mult, op1=Alu.mult, accum_out=den[:])
                rden = ev_pool.tile([128, 1], dt.float32, name="rden", tag="rden")
                nc.vector.reciprocal(rden[:], den[:])
                nc.scalar.activation(w[:], w[:], Act.Copy, bias=0.0,
                                     scale=rden[:])
                wn.append(w)

            # value-side bucket sums Wb[h][q, r] = sum_k w*eq_r
            Wb = []
            for h in range(H):
                wb = persist.tile([128, 128], dt.bfloat16, name=f"wb{h}", tag=f"wb{h}")
                Wb.append(wb)
            wbacc = [persist.tile([128, R], dt.float32, name=f"wbacc{h}", tag=f"wbacc{h}")
                     for h in range(H)]
            for r in range(R):
                for h in range(H):
                    tr = trash_pool.tile([128, S], dt.float8e4, name="tr", tag="tr")
                    nc.vector.scalar_tensor_tensor(
                        out=tr[:], in0=eq_all[:, r, :], scalar=1.0,
                        in1=wn[h][:], op0=Alu.mult, op1=Alu.mult,
                        accum_out=wbacc[h][:, r:r + 1])
            for h in range(H):
                nc.gpsimd.memset(Wb[h][:], 0.0)
                nc.scalar.copy(Wb[h][:, :R], wbacc[h][:])

            # transposes: WbT [r, q], wT [k, q] per head; then x^T on PE
            xsb = [ev_pool.tile([DK, 128], dt.bfloat16, name=f"xsb{h}", tag=f"xsb{h}")
                   for h in range(H)]
            for h in range(H):
                wbT_ps = ps_tr.tile([128, 128], dt.bfloat16, name="wbT_ps", tag="trps")
                nc.tensor.transpose(wbT_ps[:], Wb[h][:], identb[:])
                wbT = ev_pool.tile([128, 128], dt.bfloat16, name="wbT", tag="wbT")
                nc.scalar.copy(wbT[:], wbT_ps[:])

                xps = ps_x.tile([DK, 128], dt.float32, name="xps", tag="xps")
                for kc in range(KC):
                    wT_ps = ps_tr.tile([128, 128], dt.bfloat16, name="wT_ps", tag="trps")
                    nc.tensor.transpose(
                        wT_ps[:], wn[h][:, kc * 128:(kc + 1) * 128], identb[:])
                    wT = ev_pool.tile([128, 128], dt.bfloat16, name="wT", tag="wT")
                    nc.scalar.copy(wT[:], wT_ps[:])
                    nc.tensor.matmul(
                        xps[:], vv[kc][:, h * DK:(h + 1) * DK],
                        wT[:], start=(kc == 0), stop=False)
                nc.tensor.matmul(xps[:], ev_bf[:, :], wbT[:R, :],
                                 start=False, stop=True)
                nc.scalar.copy(xsb[h][:], xps[:])

            # out[q, n] = sum_D x[q, D] Wo[D, n] + bo  (8 slabs of 64)
            ops = ps_mm.tile([128, D], dt.float32, name="ops", tag="mmps")
            for h in range(H):
                nc.tensor.matmul(ops[:], xsb[h][:], Wo_sl[h][:],
                                 start=(h == 0), stop=False)
            nc.tensor.matmul(ops[:], ones_col[:], bo_row[:],
                             start=False, stop=True)
            nc.vector.tensor_copy(out_f32[:], ops[:])
            nc.sync.dma_start(out_d[qt * 128:(qt + 1) * 128, :], out_f32[:])

    nc.compile()
    return nc


def kernel(**inputs) -> np.ndarray:
    global _cached_nc
    if _cached_nc is None:
        _cached_nc = build_nc()
    nc = _cached_nc

    query = np.ascontiguousarray(inputs["query"], dtype=np.float32)
    key = np.ascontiguousarray(inputs["key"], dtype=np.float32)
    value = np.ascontiguousarray(inputs["value"], dtype=np.float32)
    mask = np.ascontiguousarray(inputs["mask"], dtype=np.int32)

    shared = {
        "Wq": np.ascontiguousarray(inputs["Wq"], np.float32),
        "Wk": np.ascontiguousarray(inputs["Wk"], np.float32),
        "Wv": np.ascontiguousarray(inputs["Wv"], np.float32),
        "Wo": np.ascontiguousarray(inputs["Wo"], np.float32),
        "bq": np.ascontiguousarray(inputs["bq"], np.float32),
        "bk": np.ascontiguousarray(inputs["bk"], np.float32),
        "bv": np.ascontiguousarray(inputs["bv"], np.float32),
        "bo": np.ascontiguousarray(inputs["bo"], np.float32),
        "emb_k": np.ascontiguousarray(inputs["emb_k"], np.float32),
        "emb_v": np.ascontiguousarray(inputs["emb_v"], np.float32),
    }

    in_maps = []
    for core in range(N_CORES):
        b, qh = core // 2, core % 2
        in_maps.append({
            "query_s": query[b, qh * QS:(qh + 1) * QS, :],
            "key_s": key[b],
            "value_s": value[b],
            "mask_s": mask[b, qh * QS:(qh + 1) * QS, :],
            **shared,
        })

    res = run_bass_kernel_spmd(nc, in_maps, core_ids=list(range(N_CORES)))

    out = np.empty((B, S, D), np.float32)
    for core in range(N_CORES):
        b, qh = core // 2, core % 2
        out[b, qh * QS:(qh + 1) * QS, :] = res.results[core]["out_s"]
    return out

